# revision 9
# baseline (speedup 1.0000x reference)
"""MoE (top-2 routed GluMLP) Trainium2 kernel, DFF-sharded over 8 NeuronCores.

Contract: kernel(**inputs) takes the FULL unsharded inputs
  x  [2, 2048, 1024] f32
  Wr [8, 1024] f32           router
  Wg [8, 4096, 1024] f32     gate proj per expert
  Wu [8, 4096, 1024] f32     up proj per expert
  Wd [8, 1024, 4096] f32     down proj per expert
and returns the FULL output [2, 2048, 1024] f32.

Strategy (DFF-parallel, perfectly load-balanced):
  - Routing (softmax + top-2 + renormalize) on host with jax on CPU using the
    exact reference ops, so selected experts / combine weights match exactly.
  - The 2*T = 8192 (token, expert) pairs are sorted by expert and processed by
    EVERY core, but each core only computes a 512-wide slice of DFF (4096/8).
    Work per core is identical regardless of routing -> no capacity padding.
  - Pairs are grouped into per-expert "blocks" of <=512 tokens (balanced sizes,
    >=344 here) so every matmul free dim is large and single-expert.
  - Per block: phase B computes h = relu(x@WgT) * (x@WuT) for the f-slice
    (fused in one DVE scalar_tensor_tensor op), then phase C immediately
    computes the partial down projection with Wd^T tiles stationary and h
    moving, writing fp16 partials [d, tokens] to DRAM. Tensor engine never
    waits between phases.
  - Host sums the 8 per-core partials, applies the combine weights, and
    scatter-adds into the final output.
  - Matmul operands fp16 (same 10-bit mantissa as TF32), fp32 PSUM accumulate:
    ~5e-4 rel err. (fp8 measured >=2.7e-2 in simulation - over the 2e-2 gate.)
"""

import math
from contextlib import ExitStack

import numpy as np

import concourse.bass as bass
import concourse.tile as tile
from concourse import bacc, mybir
from concourse.bass_utils import run_bass_kernel_spmd

B, L, D, E, TOPK, DFF = 2, 2048, 1024, 8, 2, 4096
T = B * L
PAIRS = TOPK * T        # 8192 (token, expert) pairs, all cores see all pairs
NCORES = 8
P = 128
NBT = 512               # max moving-block (one fp32 PSUM bank)
DC = D // P             # 8 contraction chunks over D
FMC = DFF // NCORES // P  # 4 f-tiles per core (512-wide DFF slice)
DT = D // P             # 8 output d-tiles
FSL = FMC * P           # 512 f per core

F32 = mybir.dt.float32
F16 = mybir.dt.float16
ALU = mybir.AluOpType

PROFILE = False
TRACE_CORES = None
LAST_EXEC_NS = None
N_WARM = 12             # warm-up matmuls to lift the HAM clock gate during DMA ramp


def _make_blocks(loads):
    """Per-expert balanced blocks of <=NBT pairs: list of (g0, ln, e)."""
    blocks, g = [], 0
    for e, ld in enumerate(loads):
        if ld == 0:
            continue
        nb = math.ceil(ld / NBT)
        base, rem = divmod(ld, nb)
        for i in range(nb):
            ln = base + (1 if i < rem else 0)
            blocks.append((g, ln, e))
            g += ln
    assert g == sum(loads)
    return blocks


def _build_nc(blocks):
    nc = bacc.Bacc(
        "TRN2",
        target_bir_lowering=False,
        debug=False,
        enable_asserts=False,
        num_devices=NCORES,
    )
    xg_d = nc.dram_tensor("xg", [P, DC, PAIRS], F16, kind="ExternalInput").ap()
    wg_d = nc.dram_tensor("wg", [E, FMC, P, DC, P], F16, kind="ExternalInput").ap()
    wu_d = nc.dram_tensor("wu", [E, FMC, P, DC, P], F16, kind="ExternalInput").ap()
    wdt_d = nc.dram_tensor("wdt", [E, P, FMC, D], F16, kind="ExternalInput").ap()
    out_d = nc.dram_tensor("out", [DT, P, PAIRS], F16, kind="ExternalOutput").ap()

    with tile.TileContext(nc) as tc:
        with ExitStack() as ctx:
            _moe_body(ctx, tc, xg_d, wg_d, wu_d, wdt_d, out_d, blocks)
    nc.compile()
    return nc


def _moe_body(ctx, tc, xg_d, wg_d, wu_d, wdt_d, out_d, blocks):
    nc = tc.nc

    xpool = ctx.enter_context(tc.tile_pool(name="xpool", bufs=4))
    hpool = ctx.enter_context(tc.tile_pool(name="hpool", bufs=3))
    wgpool = ctx.enter_context(tc.tile_pool(name="wgpool", bufs=2))
    wupool = ctx.enter_context(tc.tile_pool(name="wupool", bufs=2))
    wdtpool = ctx.enter_context(tc.tile_pool(name="wdtpool", bufs=2))
    gpool = ctx.enter_context(tc.tile_pool(name="gpool", bufs=3))
    opool = ctx.enter_context(tc.tile_pool(name="opool", bufs=3))
    warmpool = ctx.enter_context(tc.tile_pool(name="warmpool", bufs=1))
    psP = ctx.enter_context(tc.tile_pool(name="psP", bufs=8, space="PSUM"))

    # Warm-up: keep the PE busy while the first DMAs land so the HAM clock
    # gate (4/8 cold -> 8/8 warm after ~3.4us of activity) flips before the
    # real matmuls start. Operates on uninitialized SBUF - results unused.
    if N_WARM:
        warm = warmpool.tile([P, P], F16, tag="warm")
        ps_w = psP.tile([P, NBT], F32, tag="ps", name="ps_warm")
        for _ in range(N_WARM):
            nc.tensor.matmul(ps_w[:, :P], lhsT=warm, rhs=warm, start=True, stop=True)
        # dummy reader so the verifier sees the warm bank consumed
        nc.vector.tensor_copy(out=warm, in_=ps_w[:, :P])

    # x blocks stream with 2-block lookahead; one DMA per block (dma_start
    # issue costs ~0.6us of engine queue time, so batch, don't stripe)
    x_tiles = {}

    def issue_x(bi):
        if bi >= len(blocks):
            return
        g0, ln, _ = blocks[bi]
        xt_ = xpool.tile([P, DC, NBT], F16, tag="x", name=f"x{bi}")
        nc.sync.dma_start(out=xt_[:, :, :ln], in_=xg_d[:, :, g0 : g0 + ln])
        x_tiles[bi] = xt_

    # expert weight tiles, loaded one expert ahead
    wg_sb, wu_sb, wdt_sb = {}, {}, {}

    def load_expert(e, split=False):
        if e is None or e in wg_sb:
            return
        wg_t = wgpool.tile([P, FMC, DC, P], F16, tag="wg", name=f"wg{e}")
        wu_t = wupool.tile([P, FMC, DC, P], F16, tag="wu", name=f"wu{e}")
        if split:
            # first expert: per-fm loads interleaved so fm=0 lands first and
            # the first matmul can start early
            for fm in range(FMC):
                nc.scalar.dma_start(out=wg_t[:, fm], in_=wg_d[e, fm])
                nc.scalar.dma_start(out=wu_t[:, fm], in_=wu_d[e, fm])
        else:
            nc.scalar.dma_start(
                out=wg_t, in_=wg_d[e].rearrange("fm p dc f -> p fm dc f")
            )
            nc.scalar.dma_start(
                out=wu_t, in_=wu_d[e].rearrange("fm p dc f -> p fm dc f")
            )
        wd_t = wdtpool.tile([P, FMC, D], F16, tag="wdt", name=f"wdt{e}")
        nc.gpsimd.dma_start(out=wd_t, in_=wdt_d[e])
        wg_sb[e], wu_sb[e], wdt_sb[e] = wg_t, wu_t, wd_t

    eseq = []
    for _, _, e in blocks:
        if not eseq or eseq[-1] != e:
            eseq.append(e)
    enext = {e: (eseq[i + 1] if i + 1 < len(eseq) else None) for i, e in enumerate(eseq)}
    first_block_of = {}
    for bi, (_, _, e) in enumerate(blocks):
        first_block_of.setdefault(e, bi)

    load_expert(eseq[0], split=True)
    issue_x(0)
    issue_x(1)
    load_expert(enext[eseq[0]])

    for bi, (g0, ln, e) in enumerate(blocks):
        issue_x(bi + 2)
        if bi == first_block_of[e]:
            load_expert(enext[e])
        x_sb = x_tiles.pop(bi)
        h_sb = hpool.tile([P, FMC, NBT], F16, tag="h", name=f"h{bi}")

        # Phase B: h[f, t] = relu(x@WgT) * (x@WuT) for this core's f-slice
        for fm in range(FMC):
            ps_g = psP.tile([P, NBT], F32, tag="ps", name="ps_g")
            ps_u = psP.tile([P, NBT], F32, tag="ps", name="ps_u")
            for dc in range(DC):
                nc.tensor.matmul(
                    ps_g[:, :ln],
                    lhsT=wg_sb[e][:, fm, dc],
                    rhs=x_sb[:, dc, :ln],
                    start=(dc == 0),
                    stop=(dc == DC - 1),
                )
            for dc in range(DC):
                nc.tensor.matmul(
                    ps_u[:, :ln],
                    lhsT=wu_sb[e][:, fm, dc],
                    rhs=x_sb[:, dc, :ln],
                    start=(dc == 0),
                    stop=(dc == DC - 1),
                )
            g_sb = gpool.tile([P, NBT], F32, tag="g", name="g_sb")
            nc.scalar.activation(
                out=g_sb[:, :ln],
                in_=ps_g[:, :ln],
                func=mybir.ActivationFunctionType.Relu,
            )
            nc.vector.tensor_mul(h_sb[:, fm, :ln], g_sb[:, :ln], ps_u[:, :ln])

        # Phase C: out[d, t] += WdT-slice.T @ h  (WdT tiles stationary,
        # tokens moving -> no partial-tile or boundary waste). All 8 d-tiles
        # drain into one staging tile; a single batched DMA writes them out.
        o_all = opool.tile([P, DT, NBT], F16, tag="o", name=f"o{bi}")
        for dt in range(DT):
            ps_o = psP.tile([P, NBT], F32, tag="ps", name="ps_o")
            for fm in range(FMC):
                nc.tensor.matmul(
                    ps_o[:, :ln],
                    lhsT=wdt_sb[e][:, fm, dt * P : (dt + 1) * P],
                    rhs=h_sb[:, fm, :ln],
                    start=(fm == 0),
                    stop=(fm == FMC - 1),
                )
            if dt % 2 == 0:
                nc.scalar.copy(out=o_all[:, dt, :ln], in_=ps_o[:, :ln])
            else:
                nc.vector.tensor_copy(out=o_all[:, dt, :ln], in_=ps_o[:, :ln])
        nc.gpsimd.dma_start(
            out=out_d[:, :, g0 : g0 + ln].rearrange("dt p t -> p dt t"),
            in_=o_all[:, :, :ln],
        )


_NC_CACHE: dict = {}


def _get_nc(blocks):
    key = tuple(blocks)
    if key not in _NC_CACHE:
        _NC_CACHE[key] = _build_nc(blocks)
    return _NC_CACHE[key]


def _route_host(x, Wr):
    """Reference-identical routing on host (jax on CPU, same ops as reference)."""
    import jax
    import jax.numpy as jnp

    cpu = jax.devices("cpu")[0]
    with jax.default_device(cpu):
        xt = jnp.asarray(x.reshape(T, D))
        logits = jnp.einsum("td,ed->te", xt, jnp.asarray(Wr))
        scores = jax.nn.softmax(logits, axis=-1)
        k_scores, k_ids = jax.lax.top_k(scores, TOPK)
        eps = jnp.finfo(x.dtype).eps
        k_w = k_scores / (k_scores.sum(axis=-1, keepdims=True) + eps)
        return np.asarray(k_ids), np.asarray(k_w)


def kernel(x, Wr, Wg, Wu, Wd):
    global LAST_EXEC_NS
    x = np.asarray(x, dtype=np.float32)
    Wr = np.asarray(Wr, dtype=np.float32)
    Wg = np.asarray(Wg, dtype=np.float32)
    Wu = np.asarray(Wu, dtype=np.float32)
    Wd = np.asarray(Wd, dtype=np.float32)

    k_ids, k_w = _route_host(x, Wr)
    xt = x.reshape(T, D)

    tok_l, w_l, loads = [], [], []
    for e in range(E):
        tmask = k_ids == e
        tok = np.nonzero(tmask.any(axis=1))[0]
        wv = (k_w * tmask).sum(axis=1)[tok].astype(np.float32)
        tok_l.append(tok)
        w_l.append(wv)
        loads.append(len(tok))
    assert sum(loads) == PAIRS
    tok_all = np.concatenate(tok_l)
    blocks = _make_blocks(loads)

    # gathered pair inputs, device layout [p(d_inner), dc, pair]
    xg16 = xt[tok_all].astype(np.float16)
    xg_dev = np.ascontiguousarray(xg16.T.reshape(DC, P, PAIRS).transpose(1, 0, 2))

    in_maps = []
    for c in range(NCORES):
        sl = slice(c * FSL, (c + 1) * FSL)
        # Wg/Wu rows f-slice: [E, 512, D] -> [E, FMC, P(d_inner), DC, P(f)]
        wg_c = (
            Wg[:, sl, :]
            .transpose(0, 2, 1)
            .reshape(E, DC, P, FMC, P)
            .transpose(0, 3, 2, 1, 4)
            .astype(np.float16)
        )
        wu_c = (
            Wu[:, sl, :]
            .transpose(0, 2, 1)
            .reshape(E, DC, P, FMC, P)
            .transpose(0, 3, 2, 1, 4)
            .astype(np.float16)
        )
        # WdT f-slice: [E, D, 512] -> [E, P(f_inner), FMC, D]
        wdt_c = (
            Wd[:, :, sl]
            .transpose(0, 2, 1)
            .reshape(E, FMC, P, D)
            .transpose(0, 2, 1, 3)
            .astype(np.float16)
        )
        in_maps.append(
            {
                "xg": xg_dev,
                "wg": np.ascontiguousarray(wg_c),
                "wu": np.ascontiguousarray(wu_c),
                "wdt": np.ascontiguousarray(wdt_c),
            }
        )

    nc = _get_nc(blocks)
    core_ids = list(range(NCORES))
    if PROFILE:
        res = _run_profiled(nc, in_maps, core_ids)
        LAST_EXEC_NS = res.exec_time_ns
        results = res.results
    else:
        results = run_bass_kernel_spmd(nc, in_maps, core_ids).results

    # combine: sum f-slice partials, apply routing weights, scatter-add
    acc = np.zeros((D, PAIRS), dtype=np.float32)
    for c in range(NCORES):
        acc += results[c]["out"].reshape(D, PAIRS).astype(np.float32)
    accT = acc.T  # [PAIRS, D]
    out = np.zeros((T, D), dtype=np.float32)
    p0 = 0
    for e in range(E):
        ln = loads[e]
        out[tok_l[e]] += w_l[e][:, None] * accT[p0 : p0 + ln]
        p0 += ln
    return out.reshape(B, L, D)


def _run_profiled(nc, in_maps, core_ids):
    """run_bass_kernel_spmd with trace=True, providing the NTFF hook that the
    agent image's antenv stub lacks, and skipping the artifact upload."""
    import sys
    import tempfile
    import types

    import concourse.bass_utils as bu

    if "antenv.axon_hooks" not in sys.modules:
        from trn_agent_boot.trn_boot import _ntff_profile_via_ctypes

        hook = _ntff_profile_via_ctypes("/opt/axon/libaxon_pjrt.so")
        mod = types.ModuleType("antenv.axon_hooks")
        mod.get_axon_ntff_profile_hook = lambda: hook
        mod.set_axon_ntff_profile_hook = lambda h: None
        sys.modules["antenv.axon_hooks"] = mod

    orig_upload = bu.upload_artifacts
    bu.upload_artifacts = lambda tmpdir: ""
    try:
        return run_bass_kernel_spmd(
            nc,
            in_maps,
            core_ids,
            trace=True,
            trace_cores=TRACE_CORES,
            tmpdir=tempfile.mkdtemp(prefix="moe_ntff_"),
        )
    finally:
        bu.upload_artifacts = orig_upload


if __name__ == "__main__":
    # smoke test with random data (no reference comparison)
    rng = np.random.default_rng(0)
    ins = {
        "x": rng.standard_normal((B, L, D), dtype=np.float32),
        "Wr": (rng.standard_normal((E, D)) * 0.02).astype(np.float32),
        "Wg": (rng.standard_normal((E, DFF, D)) * 0.02).astype(np.float32),
        "Wu": (rng.standard_normal((E, DFF, D)) * 0.02).astype(np.float32),
        "Wd": (rng.standard_normal((E, D, DFF)) * 0.02).astype(np.float32),
    }
    out = kernel(**ins)
    print("out", out.shape, out.dtype, float(np.abs(out).max()))


# revision 15
# speedup vs baseline: 1.0621x; 1.0621x over previous
"""MoE (top-2 routed GluMLP) Trainium2 kernel, DFF-sharded over 8 NeuronCores.

Contract: kernel(**inputs) takes the FULL unsharded inputs
  x  [2, 2048, 1024] f32
  Wr [8, 1024] f32           router
  Wg [8, 4096, 1024] f32     gate proj per expert
  Wu [8, 4096, 1024] f32     up proj per expert
  Wd [8, 1024, 4096] f32     down proj per expert
and returns the FULL output [2, 2048, 1024] f32.

Strategy (DFF-parallel, perfectly load-balanced):
  - Routing (softmax + top-2 + renormalize) on host with jax on CPU using the
    exact reference ops, so selected experts / combine weights match exactly.
  - The 2*T = 8192 (token, expert) pairs are sorted by expert and processed by
    EVERY core, but each core only computes a 512-wide slice of DFF (4096/8).
    Work per core is identical regardless of routing -> no capacity padding.
  - Pairs are grouped into per-expert "blocks" of <=512 tokens (balanced sizes,
    >=344 here) so every matmul free dim is large and single-expert.
  - Per block: phase B computes h = relu(x@WgT) * (x@WuT) for the f-slice
    (fused in one DVE scalar_tensor_tensor op), then phase C immediately
    computes the partial down projection with Wd^T tiles stationary and h
    moving, writing fp16 partials [d, tokens] to DRAM. Tensor engine never
    waits between phases.
  - Host sums the 8 per-core partials, applies the combine weights, and
    scatter-adds into the final output.
  - Matmul operands fp16 (same 10-bit mantissa as TF32), fp32 PSUM accumulate:
    ~5e-4 rel err. (fp8 measured >=2.7e-2 in simulation - over the 2e-2 gate.)
"""

import math
from contextlib import ExitStack

import numpy as np

import concourse.bass as bass
import concourse.tile as tile
from concourse import bacc, mybir
from concourse.bass_utils import run_bass_kernel_spmd

B, L, D, E, TOPK, DFF = 2, 2048, 1024, 8, 2, 4096
T = B * L
PAIRS = TOPK * T        # 8192 (token, expert) pairs, all cores see all pairs
NCORES = 8
P = 128
NBT = 512               # max moving-block (one fp32 PSUM bank)
DC = D // P             # 8 contraction chunks over D
FMC = DFF // NCORES // P  # 4 f-tiles per core (512-wide DFF slice)
DT = D // P             # 8 output d-tiles
FSL = FMC * P           # 512 f per core

F32 = mybir.dt.float32
F16 = mybir.dt.float16
ALU = mybir.AluOpType

PROFILE = False
TRACE_CORES = None
LAST_EXEC_NS = None
N_WARM = 40             # warm-up matmuls to lift the HAM clock gate during DMA ramp


def _make_blocks(loads):
    """Per-expert balanced blocks of <=NBT pairs: list of (g0, ln, e)."""
    blocks, g = [], 0
    for e, ld in enumerate(loads):
        if ld == 0:
            continue
        nb = math.ceil(ld / NBT)
        base, rem = divmod(ld, nb)
        for i in range(nb):
            ln = base + (1 if i < rem else 0)
            blocks.append((g, ln, e))
            g += ln
    assert g == sum(loads)
    return blocks


def _build_nc(blocks):
    nc = bacc.Bacc(
        "TRN2",
        target_bir_lowering=False,
        debug=False,
        enable_asserts=False,
        num_devices=NCORES,
    )
    nblk = len(blocks)
    # block-major, NBT-padded layouts so every x/out DMA is one fat transfer
    # (8 KiB contiguous per partition); thin descriptors get starved by the
    # per-packet round-robin across DMA queues
    xg_d = nc.dram_tensor("xg", [nblk, P, DC, NBT], F16, kind="ExternalInput").ap()
    wg_d = nc.dram_tensor("wg", [E, FMC, P, DC, P], F16, kind="ExternalInput").ap()
    wu_d = nc.dram_tensor("wu", [E, FMC, P, DC, P], F16, kind="ExternalInput").ap()
    wdt_d = nc.dram_tensor("wdt", [E, P, FMC, D], F16, kind="ExternalInput").ap()
    out_d = nc.dram_tensor("out", [nblk, P, DT, NBT], F16, kind="ExternalOutput").ap()

    with tile.TileContext(nc) as tc:
        with ExitStack() as ctx:
            _moe_body(ctx, tc, xg_d, wg_d, wu_d, wdt_d, out_d, blocks)
    nc.compile()
    return nc


def _moe_body(ctx, tc, xg_d, wg_d, wu_d, wdt_d, out_d, blocks):
    nc = tc.nc

    xpool = ctx.enter_context(tc.tile_pool(name="xpool", bufs=4))
    hpool = ctx.enter_context(tc.tile_pool(name="hpool", bufs=3))
    wgpool = ctx.enter_context(tc.tile_pool(name="wgpool", bufs=2))
    wupool = ctx.enter_context(tc.tile_pool(name="wupool", bufs=2))
    wdtpool = ctx.enter_context(tc.tile_pool(name="wdtpool", bufs=2))
    gpool = ctx.enter_context(tc.tile_pool(name="gpool", bufs=3))
    opool = ctx.enter_context(tc.tile_pool(name="opool", bufs=3))
    warmpool = ctx.enter_context(tc.tile_pool(name="warmpool", bufs=1))
    psP = ctx.enter_context(tc.tile_pool(name="psP", bufs=8, space="PSUM"))

    # Warm-up: keep the PE busy while the first DMAs land so the HAM clock
    # gate (4/8 cold -> 8/8 warm after ~3.4us of activity) flips before the
    # real matmuls start. Operates on uninitialized SBUF - results unused.
    if N_WARM:
        warm = warmpool.tile([P, P], F16, tag="warm")
        ps_w = psP.tile([P, NBT], F32, tag="ps", name="ps_warm")
        for _ in range(N_WARM):
            nc.tensor.matmul(ps_w[:, :P], lhsT=warm, rhs=warm, start=True, stop=True)
        # dummy reader so the verifier sees the warm bank consumed
        nc.vector.tensor_copy(out=warm, in_=ps_w[:, :P])

    # x blocks stream with 2-block lookahead; one DMA per block (dma_start
    # issue costs ~0.6us of engine queue time, so batch, don't stripe)
    x_tiles = {}

    def issue_x(bi):
        if bi >= len(blocks):
            return
        xt_ = xpool.tile([P, DC, NBT], F16, tag="x", name=f"x{bi}")
        nc.sync.dma_start(out=xt_, in_=xg_d[bi])
        x_tiles[bi] = xt_

    # expert weight tiles, loaded one expert ahead
    wg_sb, wu_sb, wdt_sb = {}, {}, {}

    def load_expert(e, split=False):
        if e is None or e in wg_sb:
            return
        wg_t = wgpool.tile([P, FMC, DC, P], F16, tag="wg", name=f"wg{e}")
        wu_t = wupool.tile([P, FMC, DC, P], F16, tag="wu", name=f"wu{e}")
        if split:
            # first expert: per-fm loads interleaved so fm=0 lands first and
            # the first matmul can start early
            for fm in range(FMC):
                nc.scalar.dma_start(out=wg_t[:, fm], in_=wg_d[e, fm])
                nc.scalar.dma_start(out=wu_t[:, fm], in_=wu_d[e, fm])
        else:
            nc.scalar.dma_start(
                out=wg_t, in_=wg_d[e].rearrange("fm p dc f -> p fm dc f")
            )
            nc.scalar.dma_start(
                out=wu_t, in_=wu_d[e].rearrange("fm p dc f -> p fm dc f")
            )
        wd_t = wdtpool.tile([P, FMC, D], F16, tag="wdt", name=f"wdt{e}")
        nc.gpsimd.dma_start(out=wd_t, in_=wdt_d[e])
        wg_sb[e], wu_sb[e], wdt_sb[e] = wg_t, wu_t, wd_t

    eseq = []
    for _, _, e in blocks:
        if not eseq or eseq[-1] != e:
            eseq.append(e)
    enext = {e: (eseq[i + 1] if i + 1 < len(eseq) else None) for i, e in enumerate(eseq)}
    first_block_of = {}
    for bi, (_, _, e) in enumerate(blocks):
        first_block_of.setdefault(e, bi)

    load_expert(eseq[0], split=True)
    issue_x(0)
    issue_x(1)
    load_expert(enext[eseq[0]])

    for bi, (g0, ln, e) in enumerate(blocks):
        issue_x(bi + 2)
        if bi == first_block_of[e]:
            load_expert(enext[e])
        x_sb = x_tiles.pop(bi)
        h_sb = hpool.tile([P, FMC, NBT], F16, tag="h", name=f"h{bi}")

        # Phase B: h[f, t] = relu(x@WgT) * (x@WuT) for this core's f-slice
        for fm in range(FMC):
            ps_g = psP.tile([P, NBT], F32, tag="ps", name="ps_g")
            ps_u = psP.tile([P, NBT], F32, tag="ps", name="ps_u")
            for dc in range(DC):
                nc.tensor.matmul(
                    ps_g[:, :ln],
                    lhsT=wg_sb[e][:, fm, dc],
                    rhs=x_sb[:, dc, :ln],
                    start=(dc == 0),
                    stop=(dc == DC - 1),
                )
            for dc in range(DC):
                nc.tensor.matmul(
                    ps_u[:, :ln],
                    lhsT=wu_sb[e][:, fm, dc],
                    rhs=x_sb[:, dc, :ln],
                    start=(dc == 0),
                    stop=(dc == DC - 1),
                )
            g_sb = gpool.tile([P, NBT], F32, tag="g", name="g_sb")
            nc.scalar.activation(
                out=g_sb[:, :ln],
                in_=ps_g[:, :ln],
                func=mybir.ActivationFunctionType.Relu,
            )
            nc.vector.tensor_mul(h_sb[:, fm, :ln], g_sb[:, :ln], ps_u[:, :ln])

        # Phase C: out[d, t] += WdT-slice.T @ h  (WdT tiles stationary,
        # tokens moving -> no partial-tile or boundary waste). All 8 d-tiles
        # drain into one staging tile; a single batched DMA writes them out.
        o_all = opool.tile([P, DT, NBT], F16, tag="o", name=f"o{bi}")
        for dt in range(DT):
            ps_o = psP.tile([P, NBT], F32, tag="ps", name="ps_o")
            for fm in range(FMC):
                nc.tensor.matmul(
                    ps_o[:, :ln],
                    lhsT=wdt_sb[e][:, fm, dt * P : (dt + 1) * P],
                    rhs=h_sb[:, fm, :ln],
                    start=(fm == 0),
                    stop=(fm == FMC - 1),
                )
            if dt % 2 == 0:
                nc.scalar.copy(out=o_all[:, dt, :ln], in_=ps_o[:, :ln])
            else:
                nc.vector.tensor_copy(out=o_all[:, dt, :ln], in_=ps_o[:, :ln])
        # full-width store (pad cols are junk; host slices :ln) keeps the
        # transfer one fat contiguous run per partition
        nc.gpsimd.dma_start(out=out_d[bi], in_=o_all)


_NC_CACHE: dict = {}


def _get_nc(blocks):
    key = tuple(blocks)
    if key not in _NC_CACHE:
        _NC_CACHE[key] = _build_nc(blocks)
    return _NC_CACHE[key]


def _route_host(x, Wr):
    """Reference-identical routing on host (jax on CPU, same ops as reference)."""
    import jax
    import jax.numpy as jnp

    cpu = jax.devices("cpu")[0]
    with jax.default_device(cpu):
        xt = jnp.asarray(x.reshape(T, D))
        logits = jnp.einsum("td,ed->te", xt, jnp.asarray(Wr))
        scores = jax.nn.softmax(logits, axis=-1)
        k_scores, k_ids = jax.lax.top_k(scores, TOPK)
        eps = jnp.finfo(x.dtype).eps
        k_w = k_scores / (k_scores.sum(axis=-1, keepdims=True) + eps)
        return np.asarray(k_ids), np.asarray(k_w)


def kernel(x, Wr, Wg, Wu, Wd):
    global LAST_EXEC_NS
    x = np.asarray(x, dtype=np.float32)
    Wr = np.asarray(Wr, dtype=np.float32)
    Wg = np.asarray(Wg, dtype=np.float32)
    Wu = np.asarray(Wu, dtype=np.float32)
    Wd = np.asarray(Wd, dtype=np.float32)

    k_ids, k_w = _route_host(x, Wr)
    xt = x.reshape(T, D)

    tok_l, w_l, loads = [], [], []
    for e in range(E):
        tmask = k_ids == e
        tok = np.nonzero(tmask.any(axis=1))[0]
        wv = (k_w * tmask).sum(axis=1)[tok].astype(np.float32)
        tok_l.append(tok)
        w_l.append(wv)
        loads.append(len(tok))
    assert sum(loads) == PAIRS
    tok_all = np.concatenate(tok_l)
    blocks = _make_blocks(loads)

    # gathered pair inputs, block-major padded layout [blk, p(d_inner), dc, t]
    xg16 = xt[tok_all].astype(np.float16)
    nblk = len(blocks)
    xg_dev = np.zeros((nblk, P, DC, NBT), dtype=np.float16)
    for bi, (g0, ln, _) in enumerate(blocks):
        xb = xg16[g0 : g0 + ln]  # [ln, D]
        xg_dev[bi, :, :, :ln] = xb.T.reshape(DC, P, ln).transpose(1, 0, 2)

    in_maps = []
    for c in range(NCORES):
        sl = slice(c * FSL, (c + 1) * FSL)
        # Wg/Wu rows f-slice: [E, 512, D] -> [E, FMC, P(d_inner), DC, P(f)]
        wg_c = (
            Wg[:, sl, :]
            .transpose(0, 2, 1)
            .reshape(E, DC, P, FMC, P)
            .transpose(0, 3, 2, 1, 4)
            .astype(np.float16)
        )
        wu_c = (
            Wu[:, sl, :]
            .transpose(0, 2, 1)
            .reshape(E, DC, P, FMC, P)
            .transpose(0, 3, 2, 1, 4)
            .astype(np.float16)
        )
        # WdT f-slice: [E, D, 512] -> [E, P(f_inner), FMC, D]
        wdt_c = (
            Wd[:, :, sl]
            .transpose(0, 2, 1)
            .reshape(E, FMC, P, D)
            .transpose(0, 2, 1, 3)
            .astype(np.float16)
        )
        in_maps.append(
            {
                "xg": xg_dev,
                "wg": np.ascontiguousarray(wg_c),
                "wu": np.ascontiguousarray(wu_c),
                "wdt": np.ascontiguousarray(wdt_c),
            }
        )

    nc = _get_nc(blocks)
    core_ids = list(range(NCORES))
    if PROFILE:
        res = _run_profiled(nc, in_maps, core_ids)
        LAST_EXEC_NS = res.exec_time_ns
        results = res.results
    else:
        results = run_bass_kernel_spmd(nc, in_maps, core_ids).results

    # combine: sum f-slice partials, apply routing weights, scatter-add
    acc = np.zeros((D, PAIRS), dtype=np.float32)
    for c in range(NCORES):
        r = results[c]["out"]  # [nblk, P, DT, NBT] f16
        rt = np.ascontiguousarray(r.transpose(0, 2, 1, 3)).reshape(nblk, D, NBT)
        for bi, (g0, ln, _) in enumerate(blocks):
            acc[:, g0 : g0 + ln] += rt[bi, :, :ln]
    accT = acc.T  # [PAIRS, D]
    out = np.zeros((T, D), dtype=np.float32)
    p0 = 0
    for e in range(E):
        ln = loads[e]
        out[tok_l[e]] += w_l[e][:, None] * accT[p0 : p0 + ln]
        p0 += ln
    return out.reshape(B, L, D)


def _run_profiled(nc, in_maps, core_ids):
    """run_bass_kernel_spmd with trace=True, providing the NTFF hook that the
    agent image's antenv stub lacks, and skipping the artifact upload."""
    import sys
    import tempfile
    import types

    import concourse.bass_utils as bu

    if "antenv.axon_hooks" not in sys.modules:
        from trn_agent_boot.trn_boot import _ntff_profile_via_ctypes

        hook = _ntff_profile_via_ctypes("/opt/axon/libaxon_pjrt.so")
        mod = types.ModuleType("antenv.axon_hooks")
        mod.get_axon_ntff_profile_hook = lambda: hook
        mod.set_axon_ntff_profile_hook = lambda h: None
        sys.modules["antenv.axon_hooks"] = mod

    orig_upload = bu.upload_artifacts
    bu.upload_artifacts = lambda tmpdir: ""
    try:
        return run_bass_kernel_spmd(
            nc,
            in_maps,
            core_ids,
            trace=True,
            trace_cores=TRACE_CORES,
            tmpdir=tempfile.mkdtemp(prefix="moe_ntff_"),
        )
    finally:
        bu.upload_artifacts = orig_upload


if __name__ == "__main__":
    # smoke test with random data (no reference comparison)
    rng = np.random.default_rng(0)
    ins = {
        "x": rng.standard_normal((B, L, D), dtype=np.float32),
        "Wr": (rng.standard_normal((E, D)) * 0.02).astype(np.float32),
        "Wg": (rng.standard_normal((E, DFF, D)) * 0.02).astype(np.float32),
        "Wu": (rng.standard_normal((E, DFF, D)) * 0.02).astype(np.float32),
        "Wd": (rng.standard_normal((E, D, DFF)) * 0.02).astype(np.float32),
    }
    out = kernel(**ins)
    print("out", out.shape, out.dtype, float(np.abs(out).max()))


# revision 21
# speedup vs baseline: 1.0671x; 1.0047x over previous
"""MoE (top-2 routed GluMLP) Trainium2 kernel, DFF-sharded over 8 NeuronCores.

Contract: kernel(**inputs) takes the FULL unsharded inputs
  x  [2, 2048, 1024] f32
  Wr [8, 1024] f32           router
  Wg [8, 4096, 1024] f32     gate proj per expert
  Wu [8, 4096, 1024] f32     up proj per expert
  Wd [8, 1024, 4096] f32     down proj per expert
and returns the FULL output [2, 2048, 1024] f32.

Strategy (DFF-parallel, perfectly load-balanced):
  - Routing (softmax + top-2 + renormalize) on host with jax on CPU using the
    exact reference ops, so selected experts / combine weights match exactly.
  - The 2*T = 8192 (token, expert) pairs are sorted by expert and processed by
    EVERY core, but each core only computes a 512-wide slice of DFF (4096/8).
    Work per core is identical regardless of routing -> no capacity padding.
  - Pairs are grouped into per-expert "blocks" of <=512 tokens (balanced sizes,
    >=344 here) so every matmul free dim is large and single-expert.
  - Per block: phase B computes h = relu(x@WgT) * (x@WuT) for the f-slice
    (fused in one DVE scalar_tensor_tensor op), then phase C immediately
    computes the partial down projection with Wd^T tiles stationary and h
    moving, writing fp16 partials [d, tokens] to DRAM. Tensor engine never
    waits between phases.
  - Host sums the 8 per-core partials, applies the combine weights, and
    scatter-adds into the final output.
  - Matmul operands fp16 (same 10-bit mantissa as TF32), fp32 PSUM accumulate:
    ~5e-4 rel err. (fp8 measured >=2.7e-2 in simulation - over the 2e-2 gate.)
"""

import math
from contextlib import ExitStack

import numpy as np

import concourse.bass as bass
import concourse.tile as tile
from concourse import bacc, mybir
from concourse.bass_utils import run_bass_kernel_spmd

B, L, D, E, TOPK, DFF = 2, 2048, 1024, 8, 2, 4096
T = B * L
PAIRS = TOPK * T        # 8192 (token, expert) pairs, all cores see all pairs
NCORES = 8
P = 128
NBT = 512               # max moving-block (one fp32 PSUM bank)
DC = D // P             # 8 contraction chunks over D
FMC = DFF // NCORES // P  # 4 f-tiles per core (512-wide DFF slice)
DT = D // P             # 8 output d-tiles
FSL = FMC * P           # 512 f per core

F32 = mybir.dt.float32
F16 = mybir.dt.float16
ALU = mybir.AluOpType

PROFILE = False
TRACE_CORES = None
LAST_EXEC_NS = None
N_WARM = 28             # warm-up matmuls to lift the HAM clock gate during DMA ramp


def _make_blocks(loads):
    """Per-expert balanced blocks of <=NBT pairs: list of (g0, ln, e)."""
    blocks, g = [], 0
    for e, ld in enumerate(loads):
        if ld == 0:
            continue
        nb = math.ceil(ld / NBT)
        base, rem = divmod(ld, nb)
        for i in range(nb):
            ln = base + (1 if i < rem else 0)
            blocks.append((g, ln, e))
            g += ln
    assert g == sum(loads)
    return blocks


def _build_nc(blocks):
    nc = bacc.Bacc(
        "TRN2",
        target_bir_lowering=False,
        debug=False,
        enable_asserts=False,
        num_devices=NCORES,
    )
    nblk = len(blocks)
    # block-major, NBT-padded layouts so every x/out DMA is one fat transfer
    # (8 KiB contiguous per partition); thin descriptors get starved by the
    # per-packet round-robin across DMA queues
    xg_d = nc.dram_tensor("xg", [nblk, P, DC, NBT], F16, kind="ExternalInput").ap()
    wg_d = nc.dram_tensor("wg", [E, FMC, P, DC, P], F16, kind="ExternalInput").ap()
    wu_d = nc.dram_tensor("wu", [E, FMC, P, DC, P], F16, kind="ExternalInput").ap()
    wdt_d = nc.dram_tensor("wdt", [E, P, FMC, D], F16, kind="ExternalInput").ap()
    out_d = nc.dram_tensor("out", [nblk, P, DT, NBT], F16, kind="ExternalOutput").ap()

    with tile.TileContext(nc) as tc:
        with ExitStack() as ctx:
            _moe_body(ctx, tc, xg_d, wg_d, wu_d, wdt_d, out_d, blocks)
    nc.compile()
    return nc


def _moe_body(ctx, tc, xg_d, wg_d, wu_d, wdt_d, out_d, blocks):
    nc = tc.nc

    xpool = ctx.enter_context(tc.tile_pool(name="xpool", bufs=4))
    hpool = ctx.enter_context(tc.tile_pool(name="hpool", bufs=3))
    wgpool = ctx.enter_context(tc.tile_pool(name="wgpool", bufs=2))
    wupool = ctx.enter_context(tc.tile_pool(name="wupool", bufs=2))
    wdtpool = ctx.enter_context(tc.tile_pool(name="wdtpool", bufs=2))
    gpool = ctx.enter_context(tc.tile_pool(name="gpool", bufs=3))
    opool = ctx.enter_context(tc.tile_pool(name="opool", bufs=4))
    warmpool = ctx.enter_context(tc.tile_pool(name="warmpool", bufs=1))
    psP = ctx.enter_context(tc.tile_pool(name="psP", bufs=8, space="PSUM"))

    # Warm-up: keep the PE busy while the first DMAs land so the HAM clock
    # gate (4/8 cold -> 8/8 warm after ~3.4us of activity) flips before the
    # real matmuls start. Operates on uninitialized SBUF - results unused.
    if N_WARM:
        warm = warmpool.tile([P, P], F16, tag="warm")
        ps_w = psP.tile([P, NBT], F32, tag="ps", name="ps_warm")
        for _ in range(N_WARM):
            nc.tensor.matmul(ps_w[:, :P], lhsT=warm, rhs=warm, start=True, stop=True)
        # dummy reader so the verifier sees the warm bank consumed
        nc.vector.tensor_copy(out=warm, in_=ps_w[:, :P])

    # x blocks stream with 2-block lookahead; one DMA per block (dma_start
    # issue costs ~0.6us of engine queue time, so batch, don't stripe)
    x_tiles = {}

    def issue_x(bi, split=False):
        if bi >= len(blocks):
            return
        xt_ = xpool.tile([P, DC, NBT], F16, tag="x", name=f"x{bi}")
        if split:
            # ramp blocks: halves on both HWDGE queues so they land sooner
            nc.sync.dma_start(out=xt_[:, : DC // 2], in_=xg_d[bi, :, : DC // 2])
            nc.scalar.dma_start(out=xt_[:, DC // 2 :], in_=xg_d[bi, :, DC // 2 :])
        else:
            nc.sync.dma_start(out=xt_, in_=xg_d[bi])
        x_tiles[bi] = xt_

    # expert weight tiles, loaded one expert ahead
    wg_sb, wu_sb, wdt_sb = {}, {}, {}

    def load_expert(e, split=False):
        if e is None or e in wg_sb:
            return
        wg_t = wgpool.tile([P, FMC, DC, P], F16, tag="wg", name=f"wg{e}")
        wu_t = wupool.tile([P, FMC, DC, P], F16, tag="wu", name=f"wu{e}")
        if split:
            # first expert: per-fm loads in need-order, wg on sync / wu on
            # scalar, so fm=0 lands first and the first matmuls start early
            for fm in range(FMC):
                nc.sync.dma_start(out=wg_t[:, fm], in_=wg_d[e, fm])
                nc.scalar.dma_start(out=wu_t[:, fm], in_=wu_d[e, fm])
        else:
            nc.sync.dma_start(
                out=wg_t, in_=wg_d[e].rearrange("fm p dc f -> p fm dc f")
            )
            nc.scalar.dma_start(
                out=wu_t, in_=wu_d[e].rearrange("fm p dc f -> p fm dc f")
            )
        wd_t = wdtpool.tile([P, FMC, D], F16, tag="wdt", name=f"wdt{e}")
        nc.gpsimd.dma_start(out=wd_t, in_=wdt_d[e])
        wg_sb[e], wu_sb[e], wdt_sb[e] = wg_t, wu_t, wd_t

    eseq = []
    for _, _, e in blocks:
        if not eseq or eseq[-1] != e:
            eseq.append(e)
    enext = {e: (eseq[i + 1] if i + 1 < len(eseq) else None) for i, e in enumerate(eseq)}
    first_block_of = {}
    for bi, (_, _, e) in enumerate(blocks):
        first_block_of.setdefault(e, bi)

    issue_x(0, split=True)
    load_expert(eseq[0], split=True)
    issue_x(1, split=True)
    load_expert(enext[eseq[0]])

    for bi, (g0, ln, e) in enumerate(blocks):
        issue_x(bi + 2)
        if bi == first_block_of[e]:
            load_expert(enext[e])
        x_sb = x_tiles.pop(bi)
        h_sb = hpool.tile([P, FMC, NBT], F16, tag="h", name=f"h{bi}")

        # Phase B: h[f, t] = relu(x@WgT) * (x@WuT) for this core's f-slice
        for fm in range(FMC):
            ps_g = psP.tile([P, NBT], F32, tag="ps", name="ps_g")
            ps_u = psP.tile([P, NBT], F32, tag="ps", name="ps_u")
            for dc in range(DC):
                nc.tensor.matmul(
                    ps_g[:, :ln],
                    lhsT=wg_sb[e][:, fm, dc],
                    rhs=x_sb[:, dc, :ln],
                    start=(dc == 0),
                    stop=(dc == DC - 1),
                )
            for dc in range(DC):
                nc.tensor.matmul(
                    ps_u[:, :ln],
                    lhsT=wu_sb[e][:, fm, dc],
                    rhs=x_sb[:, dc, :ln],
                    start=(dc == 0),
                    stop=(dc == DC - 1),
                )
            g_sb = gpool.tile([P, NBT], F32, tag="g", name="g_sb")
            nc.scalar.activation(
                out=g_sb[:, :ln],
                in_=ps_g[:, :ln],
                func=mybir.ActivationFunctionType.Relu,
            )
            nc.vector.tensor_mul(h_sb[:, fm, :ln], g_sb[:, :ln], ps_u[:, :ln])

        # Phase C: out[d, t] += WdT-slice.T @ h  (WdT tiles stationary,
        # tokens moving -> no partial-tile or boundary waste). All 8 d-tiles
        # drain into one staging tile; a single batched DMA writes them out.
        o_all = opool.tile([P, DT, NBT], F16, tag="o", name=f"o{bi}")
        for dt in range(DT):
            ps_o = psP.tile([P, NBT], F32, tag="ps", name="ps_o")
            for fm in range(FMC):
                nc.tensor.matmul(
                    ps_o[:, :ln],
                    lhsT=wdt_sb[e][:, fm, dt * P : (dt + 1) * P],
                    rhs=h_sb[:, fm, :ln],
                    start=(fm == 0),
                    stop=(fm == FMC - 1),
                )
            if dt % 2 == 0:
                nc.scalar.copy(out=o_all[:, dt, :ln], in_=ps_o[:, :ln])
            else:
                nc.vector.tensor_copy(out=o_all[:, dt, :ln], in_=ps_o[:, :ln])
        # full-width store (pad cols are junk; host slices :ln) keeps the
        # transfer one fat contiguous run per partition. Last block: two
        # halves on the idle HWDGE queues so the tail drains sooner.
        if bi == len(blocks) - 1:
            nc.sync.dma_start(out=out_d[bi, :, : DT // 2], in_=o_all[:, : DT // 2])
            nc.scalar.dma_start(out=out_d[bi, :, DT // 2 :], in_=o_all[:, DT // 2 :])
        else:
            nc.gpsimd.dma_start(out=out_d[bi], in_=o_all)


_NC_CACHE: dict = {}


def _get_nc(blocks):
    key = tuple(blocks)
    if key not in _NC_CACHE:
        _NC_CACHE[key] = _build_nc(blocks)
    return _NC_CACHE[key]


def _route_host(x, Wr):
    """Reference-identical routing on host (jax on CPU, same ops as reference)."""
    import jax
    import jax.numpy as jnp

    cpu = jax.devices("cpu")[0]
    with jax.default_device(cpu):
        xt = jnp.asarray(x.reshape(T, D))
        logits = jnp.einsum("td,ed->te", xt, jnp.asarray(Wr))
        scores = jax.nn.softmax(logits, axis=-1)
        k_scores, k_ids = jax.lax.top_k(scores, TOPK)
        eps = jnp.finfo(x.dtype).eps
        k_w = k_scores / (k_scores.sum(axis=-1, keepdims=True) + eps)
        return np.asarray(k_ids), np.asarray(k_w)


def kernel(x, Wr, Wg, Wu, Wd):
    global LAST_EXEC_NS
    x = np.asarray(x, dtype=np.float32)
    Wr = np.asarray(Wr, dtype=np.float32)
    Wg = np.asarray(Wg, dtype=np.float32)
    Wu = np.asarray(Wu, dtype=np.float32)
    Wd = np.asarray(Wd, dtype=np.float32)

    k_ids, k_w = _route_host(x, Wr)
    xt = x.reshape(T, D)

    tok_l, w_l, loads = [], [], []
    for e in range(E):
        tmask = k_ids == e
        tok = np.nonzero(tmask.any(axis=1))[0]
        wv = (k_w * tmask).sum(axis=1)[tok].astype(np.float32)
        tok_l.append(tok)
        w_l.append(wv)
        loads.append(len(tok))
    assert sum(loads) == PAIRS
    tok_all = np.concatenate(tok_l)
    blocks = _make_blocks(loads)

    # gathered pair inputs, block-major padded layout [blk, p(d_inner), dc, t]
    xg16 = xt[tok_all].astype(np.float16)
    nblk = len(blocks)
    xg_dev = np.zeros((nblk, P, DC, NBT), dtype=np.float16)
    for bi, (g0, ln, _) in enumerate(blocks):
        xb = xg16[g0 : g0 + ln]  # [ln, D]
        xg_dev[bi, :, :, :ln] = xb.T.reshape(DC, P, ln).transpose(1, 0, 2)

    in_maps = []
    for c in range(NCORES):
        sl = slice(c * FSL, (c + 1) * FSL)
        # Wg/Wu rows f-slice: [E, 512, D] -> [E, FMC, P(d_inner), DC, P(f)]
        wg_c = (
            Wg[:, sl, :]
            .transpose(0, 2, 1)
            .reshape(E, DC, P, FMC, P)
            .transpose(0, 3, 2, 1, 4)
            .astype(np.float16)
        )
        wu_c = (
            Wu[:, sl, :]
            .transpose(0, 2, 1)
            .reshape(E, DC, P, FMC, P)
            .transpose(0, 3, 2, 1, 4)
            .astype(np.float16)
        )
        # WdT f-slice: [E, D, 512] -> [E, P(f_inner), FMC, D]
        wdt_c = (
            Wd[:, :, sl]
            .transpose(0, 2, 1)
            .reshape(E, FMC, P, D)
            .transpose(0, 2, 1, 3)
            .astype(np.float16)
        )
        in_maps.append(
            {
                "xg": xg_dev,
                "wg": np.ascontiguousarray(wg_c),
                "wu": np.ascontiguousarray(wu_c),
                "wdt": np.ascontiguousarray(wdt_c),
            }
        )

    nc = _get_nc(blocks)
    core_ids = list(range(NCORES))
    if PROFILE:
        res = _run_profiled(nc, in_maps, core_ids)
        LAST_EXEC_NS = res.exec_time_ns
        results = res.results
    else:
        results = run_bass_kernel_spmd(nc, in_maps, core_ids).results

    # combine: sum f-slice partials, apply routing weights, scatter-add
    acc = np.zeros((D, PAIRS), dtype=np.float32)
    for c in range(NCORES):
        r = results[c]["out"]  # [nblk, P, DT, NBT] f16
        rt = np.ascontiguousarray(r.transpose(0, 2, 1, 3)).reshape(nblk, D, NBT)
        for bi, (g0, ln, _) in enumerate(blocks):
            acc[:, g0 : g0 + ln] += rt[bi, :, :ln]
    accT = acc.T  # [PAIRS, D]
    out = np.zeros((T, D), dtype=np.float32)
    p0 = 0
    for e in range(E):
        ln = loads[e]
        out[tok_l[e]] += w_l[e][:, None] * accT[p0 : p0 + ln]
        p0 += ln
    return out.reshape(B, L, D)


def _run_profiled(nc, in_maps, core_ids):
    """run_bass_kernel_spmd with trace=True, providing the NTFF hook that the
    agent image's antenv stub lacks, and skipping the artifact upload."""
    import sys
    import tempfile
    import types

    import concourse.bass_utils as bu

    if "antenv.axon_hooks" not in sys.modules:
        from trn_agent_boot.trn_boot import _ntff_profile_via_ctypes

        hook = _ntff_profile_via_ctypes("/opt/axon/libaxon_pjrt.so")
        mod = types.ModuleType("antenv.axon_hooks")
        mod.get_axon_ntff_profile_hook = lambda: hook
        mod.set_axon_ntff_profile_hook = lambda h: None
        sys.modules["antenv.axon_hooks"] = mod

    orig_upload = bu.upload_artifacts
    bu.upload_artifacts = lambda tmpdir: ""
    try:
        return run_bass_kernel_spmd(
            nc,
            in_maps,
            core_ids,
            trace=True,
            trace_cores=TRACE_CORES,
            tmpdir=tempfile.mkdtemp(prefix="moe_ntff_"),
        )
    finally:
        bu.upload_artifacts = orig_upload


if __name__ == "__main__":
    # smoke test with random data (no reference comparison)
    rng = np.random.default_rng(0)
    ins = {
        "x": rng.standard_normal((B, L, D), dtype=np.float32),
        "Wr": (rng.standard_normal((E, D)) * 0.02).astype(np.float32),
        "Wg": (rng.standard_normal((E, DFF, D)) * 0.02).astype(np.float32),
        "Wu": (rng.standard_normal((E, DFF, D)) * 0.02).astype(np.float32),
        "Wd": (rng.standard_normal((E, D, DFF)) * 0.02).astype(np.float32),
    }
    out = kernel(**ins)
    print("out", out.shape, out.dtype, float(np.abs(out).max()))
